# revision 1
# baseline (speedup 1.0000x reference)
"""Trainium2 Bass kernel for nn_DenoiseNet_28767690949312 (denoising score loss).

Self-contained: builds an 8-core SPMD Bass kernel (one NEFF), shards inputs
data-parallel over (batch, half-of-training-points), runs via
run_bass_kernel_spmd, and combines the 8 partial sums on the host into the
scalar loss.
"""
import contextlib
import ctypes
import sys
import types

import numpy as np

sys.path.insert(0, "/opt/trn_rl_repo")

import concourse.bass as bass
import concourse.mybir as mybir
import concourse.tile as tile

F32 = mybir.dt.float32
I32 = mybir.dt.int32
U32 = mybir.dt.uint32
BF16 = mybir.dt.bfloat16
AF = mybir.ActivationFunctionType
OP = mybir.AluOpType

B, N_NOISY, N_CLEAN = 4, 4096, 4500
N_TRAIN, K_FRAME = 128, 32
DSM_SIGMA = 0.01
FEAT_KNN = 16
FE_HID, FEAT_DIM = 64, 128
HID, NUM_BLOCKS = 128, 4
QH = 64
ROWS = QH * K_FRAME
NEG = -1.0e30


def _split_multi_waits(nc, max_waits=1):
    """This walrus build rejects >1 sync wait per instruction; hoist extras
    onto standalone EventSemaphore carriers inserted before the owner."""
    for func in nc.m.functions:
        for bb in func.blocks:
            il = bb.instructions
            out = []
            changed = False
            for inst in il:
                si = inst.sync_info
                waits = list(si.on_wait) if (si is not None and si.on_wait) else []
                if len(waits) > max_waits:
                    for w in waits[:-max_waits]:
                        ev = mybir.InstEventSemaphore(
                            name=f"I-wsplit-{nc.next_id()}", ins=[], outs=[])
                        ev.engine = inst.engine
                        ev.sync_info = mybir.SyncInfo(on_wait=[w], on_update=[])
                        out.append(ev)
                    inst.sync_info = mybir.SyncInfo(
                        on_wait=waits[-max_waits:],
                        on_update=list(si.on_update) if si.on_update else [])
                    changed = True
                out.append(inst)
            if changed:
                bb.instructions = out


def _bcast_qk(ap_2d, q0, nq, nk):
    sl = ap_2d[:, q0:q0 + nq]
    return bass.AP(sl.tensor, sl.offset, [sl.ap[0], [sl.ap[-1][0], nq], [0, nk]])


def _build():
    nc = bass.Bass()
    noisy = nc.dram_tensor("noisy", [N_NOISY, 3], F32, kind="ExternalInput")
    noisyT = nc.dram_tensor("noisyT", [3, N_NOISY], F32, kind="ExternalInput")
    cleanT = nc.dram_tensor("cleanT", [3, N_CLEAN], F32, kind="ExternalInput")
    pidx = nc.dram_tensor("pidx", [QH], I32, kind="ExternalInput")
    ident_in = nc.dram_tensor("ident", [128, 128], F32, kind="ExternalInput")
    feW1 = nc.dram_tensor("feW1", [6, FE_HID], F32, kind="ExternalInput")
    feb1 = nc.dram_tensor("feb1", [FE_HID], F32, kind="ExternalInput")
    feW2 = nc.dram_tensor("feW2", [FE_HID, FEAT_DIM], F32, kind="ExternalInput")
    feb2 = nc.dram_tensor("feb2", [FEAT_DIM], F32, kind="ExternalInput")
    Wp = nc.dram_tensor("Wp", [FEAT_DIM + 3, HID], F32, kind="ExternalInput")
    bp = nc.dram_tensor("bp", [HID], F32, kind="ExternalInput")
    g0 = nc.dram_tensor("g0", [NUM_BLOCKS, HID], F32, kind="ExternalInput")
    b0 = nc.dram_tensor("b0", [NUM_BLOCKS, HID], F32, kind="ExternalInput")
    W0 = nc.dram_tensor("W0", [NUM_BLOCKS, HID, HID], F32, kind="ExternalInput")
    bf0 = nc.dram_tensor("bf0", [NUM_BLOCKS, HID], F32, kind="ExternalInput")
    g1 = nc.dram_tensor("g1", [NUM_BLOCKS, HID], F32, kind="ExternalInput")
    b1 = nc.dram_tensor("b1", [NUM_BLOCKS, HID], F32, kind="ExternalInput")
    W1 = nc.dram_tensor("W1", [NUM_BLOCKS, HID, HID], F32, kind="ExternalInput")
    bf1 = nc.dram_tensor("bf1", [NUM_BLOCKS, HID], F32, kind="ExternalInput")
    Wc = nc.dram_tensor("Wc", [NUM_BLOCKS, FEAT_DIM + 3, HID], F32, kind="ExternalInput")
    bc = nc.dram_tensor("bc", [NUM_BLOCKS, HID], F32, kind="ExternalInput")
    gO = nc.dram_tensor("gO", [HID], F32, kind="ExternalInput")
    bO = nc.dram_tensor("bO", [HID], F32, kind="ExternalInput")
    Wo = nc.dram_tensor("Wo", [HID, 1], F32, kind="ExternalInput")
    bo = nc.dram_tensor("bo", [1], F32, kind="ExternalInput")
    partial = nc.dram_tensor("partial", [1, 1], F32, kind="ExternalOutput")

    CH_D = [512] * 8 + [404]
    CH_N = [512] * 8

    with tile.TileContext(nc) as tc, contextlib.ExitStack() as ctx:
        E = ctx.enter_context
        con = E(tc.tile_pool(name="con", bufs=1))
        big = E(tc.tile_pool(name="big", bufs=1))
        sc = E(tc.tile_pool(name="sc", bufs=2))
        dp = E(tc.tile_pool(name="dp", bufs=2))  # shared dist work tiles
        ps_big = E(tc.tile_pool(name="psb", bufs=2, space="PSUM"))
        ps_sm = E(tc.tile_pool(name="psm", bufs=4, space="PSUM"))

        ident = con.tile([128, 128], F32)
        nc.sync.dma_start(ident[:], ident_in[:])
        ones = con.tile([128, 1], F32)
        nc.vector.memset(ones[:], 1.0)
        ones_r = con.tile([1, 128], F32)
        nc.vector.memset(ones_r[:], 1.0)
        neg2_128 = con.tile([2, 128], F32)
        nc.vector.memset(neg2_128[:], -1.0)
        neg3_64 = con.tile([3, QH], F32)
        nc.vector.memset(neg3_64[:], -1.0)

        _vec_n = [0]

        def load_vec(src_ap, n):
            _vec_n[0] += 1
            t = con.tile([n, 1], F32, name=f"v{_vec_n[0]}", tag=f"v{_vec_n[0]}")
            nc.sync.dma_start(t[:], bass.AP(src_ap.tensor, src_ap.offset,
                                            [src_ap.ap[-1], [1, 1]]))
            return t

        feb1_t = load_vec(feb1[:], FE_HID)
        feb2_t = load_vec(feb2[:], FEAT_DIM)
        bp_t = load_vec(bp[:], HID)
        gO_t = load_vec(gO[:], HID)
        bO_t = load_vec(bO[:], HID)
        bo_t = con.tile([128, 1], F32)
        nc.sync.dma_start(bo_t[:], bass.AP(bo, 0, [[0, 128], [1, 1]]))

        g0_t, b0_t, g1_t, comb1_t, comb2_t = [], [], [], [], []
        for i in range(NUM_BLOCKS):
            g0_t.append(load_vec(g0[i], HID))
            b0_t.append(load_vec(b0[i], HID))
            g1_t.append(load_vec(g1[i], HID))
            bf0_i = load_vec(bf0[i], HID)
            b1_i = load_vec(b1[i], HID)
            c1 = con.tile([HID, 1], F32, tag=f"c1_{i}")
            nc.vector.tensor_tensor(out=c1[:], in0=g1_t[i][:], in1=bf0_i[:], op=OP.mult)
            nc.vector.tensor_tensor(out=c1[:], in0=c1[:], in1=b1_i[:], op=OP.add)
            comb1_t.append(c1)
            bf1_i = load_vec(bf1[i], HID)
            bc_i = load_vec(bc[i], HID)
            c2 = con.tile([HID, 1], F32, tag=f"c2_{i}")
            nc.vector.tensor_tensor(out=c2[:], in0=bf1_i[:], in1=bc_i[:], op=OP.add)
            comb2_t.append(c2)

        def load_mat(src, k, m, tag):
            t = con.tile([k, m], F32, tag=tag)
            nc.sync.dma_start(t[:], src)
            return t

        feW1a = load_mat(feW1[0:3, :], 3, FE_HID, "feW1a")
        feW1b = load_mat(feW1[3:6, :], 3, FE_HID, "feW1b")
        feW2_t = load_mat(feW2[:], FE_HID, FEAT_DIM, "feW2")
        Wp_x = load_mat(Wp[0:3, :], 3, HID, "Wpx")
        Wp_f = load_mat(Wp[3:, :], FEAT_DIM, HID, "Wpf")
        W0_t = [load_mat(W0[i], HID, HID, f"W0_{i}") for i in range(NUM_BLOCKS)]
        W1_t = [load_mat(W1[i], HID, HID, f"W1_{i}") for i in range(NUM_BLOCKS)]
        Wc_x = [load_mat(Wc[i, 0:3, :], 3, HID, f"Wcx_{i}") for i in range(NUM_BLOCKS)]
        Wc_f = [load_mat(Wc[i, 3:, :], FEAT_DIM, HID, f"Wcf_{i}") for i in range(NUM_BLOCKS)]
        Wo_t = load_mat(Wo[:], HID, 1, "Wo")

        def to_bf16(src, k, m, tag):
            t = con.tile([k, m], BF16, name=tag, tag=tag)
            nc.vector.tensor_copy(t[:], src[:])
            return t

        Wp_xb = to_bf16(Wp_x, 3, HID, "Wpxb")
        Wp_fb = to_bf16(Wp_f, FEAT_DIM, HID, "Wpfb")
        W0_b = [to_bf16(W0_t[i], HID, HID, f"W0b_{i}") for i in range(NUM_BLOCKS)]
        W1_b = [to_bf16(W1_t[i], HID, HID, f"W1b_{i}") for i in range(NUM_BLOCKS)]
        Wc_xb = [to_bf16(Wc_x[i], 3, HID, f"Wcxb_{i}") for i in range(NUM_BLOCKS)]
        Wc_fb = [to_bf16(Wc_f[i], FEAT_DIM, HID, f"Wcfb_{i}") for i in range(NUM_BLOCKS)]
        Wo_b = to_bf16(Wo_t, HID, 1, "Wob")

        nT = big.tile([3, N_NOISY], F32)
        nc.sync.dma_start(nT[:], noisyT[:])
        nsq = dp.tile([3, N_NOISY], F32, tag="dist", name="nsq")
        nc.scalar.activation(nsq[:], nT[:], AF.Square)
        # interleaved ref planes: rows (x, y, x^2, y^2, z, z^2)
        rhs6n = big.tile([6, N_NOISY], F32)
        nc.sync.dma_start(rhs6n[0:2, :], nT[0:2, :])
        nc.sync.dma_start(rhs6n[2:4, :], nsq[0:2, :])
        nc.sync.dma_start(rhs6n[4:5, :], nT[2:3, :])
        nc.sync.dma_start(rhs6n[5:6, :], nsq[2:3, :])
        cT = big.tile([3, N_CLEAN], F32)
        nc.sync.dma_start(cT[:], cleanT[:])
        csq = dp.tile([2, N_CLEAN], F32, tag="dist", name="csq")
        nc.scalar.activation(csq[:], cT[0:2, :], AF.Square)
        rhs4c = big.tile([4, N_CLEAN], F32)
        nc.sync.dma_start(rhs4c[0:2, :], cT[0:2, :])
        nc.sync.dma_start(rhs4c[2:4, :], csq[:])
        cz1 = big.tile([1, N_CLEAN], F32)
        nc.sync.dma_start(cz1[:], cT[2:3, :])
        cleanzB = big.tile([128, N_CLEAN], F32)
        for ci, w in enumerate(CH_D):
            o = 512 * ci
            pz = ps_big.tile([128, 512], F32, tag="pd")
            nc.tensor.matmul(pz[:, :w], ones_r[:], cz1[0:1, o:o + w], start=True, stop=True)
            nc.scalar.copy(cleanzB[:, o:o + w], pz[:, :w])

        pidx_t = con.tile([QH, 1], I32)
        nc.sync.dma_start(pidx_t[:], bass.AP(pidx, 0, [[1, QH], [1, 1]]))
        ctr = con.tile([QH, 3], F32)
        nc.gpsimd.indirect_dma_start(
            out=ctr[:], out_offset=None, in_=noisy[:],
            in_offset=bass.IndirectOffsetOnAxis(ap=pidx_t[:, :1], axis=0))
        ctrT_ps = ps_sm.tile([3, QH], F32, tag="pt")
        nc.tensor.transpose(ctrT_ps[:], ctr[:, 0:3], ident[0:QH, 0:QH])
        ctrT = con.tile([3, QH], F32)
        nc.scalar.copy(ctrT[:], ctrT_ps[:])
        q2x3 = con.tile([3, QH], F32)
        nc.scalar.mul(q2x3[:], ctrT[:], 2.0)
        q6 = con.tile([6, QH], F32)
        nc.vector.memset(q6[:], -1.0)
        nc.sync.dma_start(q6[0:2, :], q2x3[0:2, :])
        nc.sync.dma_start(q6[4:5, :], q2x3[2:3, :])

        # feature net: 3D knn k=16
        distA = dp.tile([QH, N_NOISY], F32, tag="dist")
        for ci, w in enumerate(CH_N):
            o = 512 * ci
            pd = ps_big.tile([QH, 512], F32, tag="pd")
            nc.tensor.matmul(pd[:, :w], q6[:], rhs6n[:, o:o + w], start=True, stop=True)
            nc.scalar.copy(distA[:, o:o + w], pd[:, :w])
        idx16 = con.tile([QH, 16], U32)
        mxB = sc.tile([QH, 8], F32, tag="mx8")
        nc.vector.max(out=mxB[:], in_=distA[:])
        nc.vector.max_index(out=idx16[:, 0:8], in_max=mxB[:], in_values=distA[:])
        distB = dp.tile([QH, N_NOISY], F32, tag="dist")
        nc.vector.match_replace(out=distB[:], in_to_replace=mxB[:], in_values=distA[:], imm_value=NEG)
        mxB2 = sc.tile([QH, 8], F32, tag="mx8")
        nc.vector.max(out=mxB2[:], in_=distB[:])
        nc.vector.max_index(out=idx16[:, 8:16], in_max=mxB2[:], in_values=distB[:])

        h_acc = con.tile([FE_HID, QH], F32)
        nc.vector.memset(h_acc[:], 0.0)
        for k in range(FEAT_KNN):
            nb = sc.tile([QH, 3], F32, tag="nb")
            nc.gpsimd.indirect_dma_start(
                out=nb[:], out_offset=None, in_=noisy[:],
                in_offset=bass.IndirectOffsetOnAxis(ap=idx16[:, k:k + 1].bitcast(I32), axis=0))
            nbT_ps = ps_sm.tile([3, QH], F32, tag="pt")
            nc.tensor.transpose(nbT_ps[:], nb[:, 0:3], ident[0:QH, 0:QH])
            dif = sc.tile([3, QH], F32, tag="dif")
            nc.vector.tensor_tensor(out=dif[:], in0=nbT_ps[:], in1=ctrT[:], op=OP.subtract)
            hp = ps_sm.tile([FE_HID, QH], F32, tag="pt")
            nc.tensor.matmul(hp[:], feW1a[:], ctrT[:], start=True, stop=False)
            nc.tensor.matmul(hp[:], feW1b[:], dif[:], start=False, stop=True)
            nc.vector.scalar_tensor_tensor(
                out=h_acc[:], in0=hp[:], scalar=feb1_t[:, :1], in1=h_acc[:],
                op0=OP.add, op1=OP.max)
        fps = ps_sm.tile([FEAT_DIM, QH], F32, tag="pt")
        nc.tensor.matmul(fps[:], feW2_t[:], h_acc[:], start=True, stop=True)
        featT = con.tile([FEAT_DIM, QH], BF16)
        nc.scalar.activation(featT[:], fps[:], AF.Relu, bias=feb2_t[:, :1], scale=1.0)

        # frame knn: xy k=32
        distC = dp.tile([QH, N_NOISY], F32, tag="dist")
        for ci, w in enumerate(CH_N):
            o = 512 * ci
            pd = ps_big.tile([QH, 512], F32, tag="pd")
            nc.tensor.matmul(pd[:, :w], q6[0:4, :], rhs6n[0:4, o:o + w], start=True, stop=True)
            nc.scalar.copy(distC[:, o:o + w], pd[:, :w])
        nn32 = con.tile([QH, 32], U32)
        cur = distC
        for r in range(4):
            mx = sc.tile([QH, 8], F32, tag="mx8")
            nc.vector.max(out=mx[:], in_=cur[:])
            nc.vector.max_index(out=nn32[:, 8 * r:8 * r + 8], in_max=mx[:], in_values=cur[:])
            if r < 3:
                nxt = dp.tile([QH, N_NOISY], F32, tag="dist")
                nc.vector.match_replace(out=nxt[:], in_to_replace=mx[:], in_values=cur[:], imm_value=NEG)
                cur = nxt

        xyzT = big.tile([3, ROWS], BF16)
        q4f = big.tile([4, ROWS], F32)
        nc.vector.memset(q4f[:], -1.0)
        zf = con.tile([128, 16], F32)
        gt = con.tile([128, 16], F32)
        idxt_all = con.tile([128, 16], I32)
        for t in range(16):
            it = idxt_all[:, t:t + 1]
            nc.sync.dma_start(it, nn32[4 * t:4 * t + 4, :].bitcast(I32))
            fr = sc.tile([128, 3], F32, tag="fr")
            nc.gpsimd.indirect_dma_start(
                out=fr[:], out_offset=None, in_=noisy[:],
                in_offset=bass.IndirectOffsetOnAxis(ap=it, axis=0))
            nc.vector.tensor_copy(zf[:, t:t + 1], fr[:, 2:3])
            frT_ps = ps_sm.tile([3, 128], F32, tag="pt")
            nc.tensor.transpose(frT_ps[:], fr[:, 0:3], ident[:])
            nc.vector.tensor_tensor(
                out=xyzT[:, 128 * t:128 * t + 128], in0=frT_ps[:],
                in1=_bcast_qk(ctrT, 4 * t, 4, 32), op=OP.subtract)
            nc.scalar.activation(q4f[0:2, 128 * t:128 * t + 128], frT_ps[0:2, :], AF.Copy, scale=2.0)

        # clean knn top-4 masked z-mean
        for t in range(16):
            distD = dp.tile([128, N_CLEAN], F32, tag="dist")
            for ci, w in enumerate(CH_D):
                o = 512 * ci
                pd = ps_big.tile([128, 512], F32, tag="pd")
                nc.tensor.matmul(pd[:, :w], q4f[:, 128 * t:128 * t + 128], rhs4c[:, o:o + w], start=True, stop=True)
                nc.scalar.copy(distD[:, o:o + w], pd[:, :w])
            mxD = sc.tile([128, 8], F32, tag="mx8")
            nc.vector.max(out=mxD[:], in_=distD[:])
            zsum = sc.tile([128, 1], F32, tag="zsum")
            nc.vector.scalar_tensor_tensor(
                out=distD[:], in0=distD[:], scalar=mxD[:, 3:4], in1=cleanzB[:],
                op0=OP.is_ge, op1=OP.mult, accum_out=zsum[:])
            nc.vector.scalar_tensor_tensor(
                out=gt[:, t:t + 1], in0=zsum[:], scalar=0.25, in1=zf[:, t:t + 1],
                op0=OP.mult, op1=OP.subtract)

        # score net + loss
        lacc = con.tile([128, 1], F32)
        nc.vector.memset(lacc[:], 0.0)
        for blk in range(4):
            r0 = 512 * blk
            xyz_b = xyzT[:, r0:r0 + 512]
            feat_b = _bcast_qk(featT, 16 * blk, 16, 32)
            pA = ps_big.tile([HID, 512], F32, tag="ps")
            nc.tensor.matmul(pA[:], Wp_xb[:], xyz_b, start=True, stop=False)
            nc.tensor.matmul(pA[:], Wp_fb[:], feat_b, start=False, stop=True)
            net = sc.tile([HID, 512], F32, tag="net")
            nc.scalar.activation(net[:], pA[:], AF.Identity, bias=bp_t[:, :1], scale=1.0)
            for i in range(NUM_BLOCKS):
                h1 = sc.tile([HID, 512], BF16, tag="h1")
                nc.scalar.activation(h1[:], net[:], AF.Relu, bias=b0_t[i][:, :1], scale=g0_t[i][:, :1])
                pB = ps_big.tile([HID, 512], F32, tag="ps")
                nc.tensor.matmul(pB[:], W0_b[i][:], h1[:], start=True, stop=True)
                h2 = sc.tile([HID, 512], BF16, tag="h1")
                nc.scalar.activation(h2[:], pB[:], AF.Relu, bias=comb1_t[i][:, :1], scale=g1_t[i][:, :1])
                pC = ps_big.tile([HID, 512], F32, tag="ps")
                nc.tensor.matmul(pC[:], W1_b[i][:], h2[:], start=True, stop=False)
                nc.tensor.matmul(pC[:], Wc_xb[i][:], xyz_b, start=False, stop=False)
                nc.tensor.matmul(pC[:], Wc_fb[i][:], feat_b, start=False, stop=True)
                nc.vector.scalar_tensor_tensor(
                    out=net[:], in0=pC[:], scalar=comb2_t[i][:, :1], in1=net[:],
                    op0=OP.add, op1=OP.add)
            rfin = sc.tile([HID, 512], BF16, tag="h1")
            nc.scalar.activation(rfin[:], net[:], AF.Relu, bias=bO_t[:, :1], scale=gO_t[:, :1])
            for j in range(4):
                t = 4 * blk + j
                gp_ps = ps_sm.tile([128, 1], F32, tag="pt")
                nc.tensor.matmul(gp_ps[:], rfin[:, 128 * j:128 * j + 128], Wo_b[:], start=True, stop=True)
                diff = sc.tile([128, 1], F32, tag="diff")
                nc.vector.scalar_tensor_tensor(
                    out=diff[:], in0=gp_ps[:], scalar=bo_t[:, :1], in1=gt[:, t:t + 1],
                    op0=OP.add, op1=OP.subtract)
                sq = sc.tile([128, 1], F32, tag="sq")
                nc.vector.tensor_tensor(out=sq[:], in0=diff[:], in1=diff[:], op=OP.mult)
                nc.vector.tensor_tensor(out=lacc[:], in0=lacc[:], in1=sq[:], op=OP.add)

        lps = ps_sm.tile([1, 1], F32, tag="pt")
        nc.tensor.matmul(lps[:], lacc[:], ones[:], start=True, stop=True)
        lsb = con.tile([1, 1], F32)
        nc.scalar.copy(lsb[:], lps[:])
        nc.sync.dma_start(partial[:], lsb[:])

    _split_multi_waits(nc)
    return nc


_NC_CACHE = {}


def _get_nc():
    if "nc" not in _NC_CACHE:
        _NC_CACHE["nc"] = _build()
    return _NC_CACHE["nc"]


def kernel(**inputs) -> np.ndarray:
    from concourse.bass_utils import run_bass_kernel_spmd

    pcl_noisy = np.ascontiguousarray(np.asarray(inputs["pcl_noisy"], np.float32))
    pcl_clean = np.ascontiguousarray(np.asarray(inputs["pcl_clean"], np.float32))
    pnt_idx = np.asarray(inputs["pnt_idx"]).astype(np.int32)
    common = {"ident": np.eye(128, dtype=np.float32)}
    for k in ("feW1", "feb1", "feW2", "feb2", "Wp", "bp", "g0", "b0", "W0", "bf0",
              "g1", "b1", "W1", "bf1", "Wc", "bc", "gO", "bO", "Wo", "bo"):
        common[k] = np.ascontiguousarray(np.asarray(inputs[k], np.float32))
    in_maps = []
    for core in range(8):
        b, h = core // 2, core % 2
        m = dict(common)
        m["noisy"] = pcl_noisy[b]
        m["noisyT"] = np.ascontiguousarray(pcl_noisy[b].T)
        m["cleanT"] = np.ascontiguousarray(pcl_clean[b].T)
        m["pidx"] = np.ascontiguousarray(pnt_idx[64 * h:64 * h + 64])
        in_maps.append(m)

    res = run_bass_kernel_spmd(_get_nc(), in_maps, core_ids=list(range(8)))
    total = float(np.sum([np.asarray(res.results[i]["partial"]).reshape(())
                          for i in range(8)]))
    loss = 0.5 * total / (B * N_TRAIN * K_FRAME) / DSM_SIGMA
    return np.float32(loss)



# revision 2
# speedup vs baseline: 1.0272x; 1.0272x over previous
"""Trainium2 Bass kernel for nn_DenoiseNet_28767690949312 (denoising score loss).

Per-core layout (core = 2*b + h): batch b, queries q in [64h, 64h+64).
Key techniques vs v1:
- all distance matmuls in bf16 6-term split products (host-precomputed
  split planes shipped as uint16 bf16-bits), fp32 PSUM accumulation;
  per-query centering folded into the matmul as extra contract rows.
- noisy KNNs (feat k=16, frame k=32) in a dup-half layout [128, 2048]
  with candidate indices embedded in the low 12 mantissa bits, so
  selection is MAX8 + MATCH_REPLACE rounds only (no FIND_INDEX8),
  followed by a cheap [64, 2k] merge.
- clean KNN: one [128,24] indirect gather + one transpose per t-chunk
  yields the per-frame-point stationary rows (incl. centering rows);
  threshold MAX8 + masked-z STT stay fp32 on Vector.
"""
import contextlib
import sys

import numpy as np

sys.path.insert(0, "/opt/trn_rl_repo")

import concourse.bass as bass
import concourse.mybir as mybir
import concourse.tile as tile

F32 = mybir.dt.float32
I32 = mybir.dt.int32
U32 = mybir.dt.uint32
U16 = mybir.dt.uint16
BF16 = mybir.dt.bfloat16
AF = mybir.ActivationFunctionType
OP = mybir.AluOpType

B, N_NOISY, N_CLEAN = 4, 4096, 4500
N_TRAIN, K_FRAME = 128, 32
DSM_SIGMA = 0.01
FE_HID, FEAT_DIM = 64, 128
HID, NUM_BLOCKS = 128, 4
QH = 64
HALF = 2048
NEG = -1.0e30

# contract-row counts
CA_H = 24            # feat per-half rows: 3 dims * 6 + c2(3) + center(3)
CA = 2 * CA_H        # 48
CC_H = 18            # frame per-half rows: 2 dims * 6 + 3 + 3
CC = 2 * CC_H        # 36
CD = 18              # clean rows: 2 dims * 6 + c2(3) + center(3)
NTAB_W = 24          # ntab cols: 0:3 xyz, 3 pad, 4:22 lhs rows, 22:24 pad
DEBUG = False        # adds idx16/nn32/gt debug outputs


def _split_multi_waits(nc, max_waits=1):
    for func in nc.m.functions:
        for bb in func.blocks:
            il = bb.instructions
            out = []
            changed = False
            for inst in il:
                si = inst.sync_info
                waits = list(si.on_wait) if (si is not None and si.on_wait) else []
                if len(waits) > max_waits:
                    for w in waits[:-max_waits]:
                        ev = mybir.InstEventSemaphore(
                            name=f"I-wsplit-{nc.next_id()}", ins=[], outs=[])
                        ev.engine = inst.engine
                        ev.sync_info = mybir.SyncInfo(on_wait=[w], on_update=[])
                        out.append(ev)
                    inst.sync_info = mybir.SyncInfo(
                        on_wait=waits[-max_waits:],
                        on_update=list(si.on_update) if si.on_update else [])
                    changed = True
                out.append(inst)
            if changed:
                bb.instructions = out


def _bcast_qk(ap_2d, q0, nq, nk):
    sl = ap_2d[:, q0:q0 + nq]
    return bass.AP(sl.tensor, sl.offset, [sl.ap[0], [sl.ap[-1][0], nq], [0, nk]])


def _build():
    nc = bass.Bass()
    noisy = nc.dram_tensor("noisy", [N_NOISY, 3], F32, kind="ExternalInput")
    ntab = nc.dram_tensor("ntab", [N_NOISY, NTAB_W], F32, kind="ExternalInput")
    rhsA = nc.dram_tensor("rhsA", [CA, HALF], U16, kind="ExternalInput")
    lhsA = nc.dram_tensor("lhsA", [CA, 128], U16, kind="ExternalInput")
    rhsC = nc.dram_tensor("rhsC", [CC, HALF], U16, kind="ExternalInput")
    lhsC = nc.dram_tensor("lhsC", [CC, 128], U16, kind="ExternalInput")
    cplanes = nc.dram_tensor("cplanes", [CD, N_CLEAN], U16, kind="ExternalInput")
    ebits_in = nc.dram_tensor("ebits", [128, HALF], I32, kind="ExternalInput")
    maskc_in = nc.dram_tensor("maskc", [128, 2], I32, kind="ExternalInput")
    czb_in = nc.dram_tensor("czb", [128, N_CLEAN], F32, kind="ExternalInput")
    ctrTk_in = nc.dram_tensor("ctrTk", [3, QH], F32, kind="ExternalInput")
    ctr2T_in = nc.dram_tensor("ctr2T", [3, 128], F32, kind="ExternalInput")
    ident_in = nc.dram_tensor("ident", [128, 128], F32, kind="ExternalInput")
    # packed weight blobs (host-packed):
    # wbf cols: Wp_f(128) W0[0..3](512) W1[0..3](512) Wc_f[0..3](512) Wo(1)
    wbf_in = nc.dram_tensor("wbf", [128, 1665], U16, kind="ExternalInput")
    # xbf cols: Wp_x(128) Wc_x[0..3](512)
    xbf_in = nc.dram_tensor("xbf", [3, 640], U16, kind="ExternalInput")
    few2_in = nc.dram_tensor("few2b", [FE_HID, FEAT_DIM], U16, kind="ExternalInput")
    feW1 = nc.dram_tensor("feW1", [6, FE_HID], F32, kind="ExternalInput")
    # bias cols: bp g0x4 b0x4 g1x4 comb1x4 comb2x4 gO bO bo feb1 feb2 = 26
    bias_in = nc.dram_tensor("biasb", [128, 26], F32, kind="ExternalInput")
    partial = nc.dram_tensor("partial", [1, 1], F32, kind="ExternalOutput")
    if DEBUG:
        idx16_d = nc.dram_tensor("idx16_d", [QH, 16], I32, kind="ExternalOutput")
        nn32_d = nc.dram_tensor("nn32_d", [QH, 32], I32, kind="ExternalOutput")
        gt_d = nc.dram_tensor("gt_d", [128, 16], F32, kind="ExternalOutput")
        feat_d = nc.dram_tensor("feat_d", [FEAT_DIM, QH], F32, kind="ExternalOutput")

    with tile.TileContext(nc) as tc, contextlib.ExitStack() as ctx:
        E = ctx.enter_context
        con = E(tc.tile_pool(name="con", bufs=1))
        big = E(tc.tile_pool(name="big", bufs=1))
        sc = E(tc.tile_pool(name="sc", bufs=2))
        dp = E(tc.tile_pool(name="dp", bufs=2))      # distD fp32 rows
        ps_d = E(tc.tile_pool(name="psd", bufs=2, space="PSUM"))   # [128,1024]
        ps_net = E(tc.tile_pool(name="psn", bufs=2, space="PSUM"))  # [128,512]
        ps_sm = E(tc.tile_pool(name="pss", bufs=2, space="PSUM"))   # small

        # ---------- constant loads (knn-phase inputs first) ----------
        ident = con.tile([128, 128], F32)
        nc.sync.dma_start(ident[:], ident_in[:])
        maskc = con.tile([128, 2], I32)
        nc.sync.dma_start(maskc[:], maskc_in[:])
        ebits = big.tile([128, HALF], I32)
        nc.sync.dma_start(ebits[:], ebits_in[:])
        rhsA_t = big.tile([CA, HALF], U16)
        nc.sync.dma_start(rhsA_t[:], rhsA[:])
        lhsA_t = con.tile([CA, 128], U16)
        nc.sync.dma_start(lhsA_t[:], lhsA[:])
        rhsC_t = big.tile([CC, HALF], U16)
        nc.sync.dma_start(rhsC_t[:], rhsC[:])
        lhsC_t = con.tile([CC, 128], U16)
        nc.sync.dma_start(lhsC_t[:], lhsC[:])
        ctr2T = con.tile([3, 128], F32)
        nc.sync.dma_start(ctr2T[:], ctr2T_in[:])
        ctrTk = con.tile([3, QH], F32)
        nc.sync.dma_start(ctrTk[:], ctrTk_in[:])
        feW1a_t = con.tile([3, FE_HID], F32)
        nc.sync.dma_start(feW1a_t[:], feW1[0:3, :])
        feW1b_t = con.tile([3, FE_HID], F32)
        nc.sync.dma_start(feW1b_t[:], feW1[3:6, :])
        feW1a, feW1b = feW1a_t[:], feW1b_t[:]
        feW2_b = con.tile([FE_HID, FEAT_DIM], U16)
        nc.sync.dma_start(feW2_b[:], few2_in[:])
        cpl = big.tile([CD, N_CLEAN], U16)
        nc.sync.dma_start(cpl[:], cplanes[:])
        czb = big.tile([128, N_CLEAN], F32)
        nc.sync.dma_start(czb[:], czb_in[:])
        wbf = big.tile([128, 1665], U16)
        nc.sync.dma_start(wbf[:], wbf_in[:])
        xbf = con.tile([3, 640], U16)
        nc.sync.dma_start(xbf[:], xbf_in[:])
        biasb = con.tile([128, 26], F32)
        nc.sync.dma_start(biasb[:], bias_in[:])
        ones = con.tile([128, 1], F32)
        nc.vector.memset(ones[:], 1.0)
        izer = con.tile([128, 64], I32)
        nc.vector.memset(izer[:], 0.0)

        # weight slices (bf16 bit views)
        Wp_fb = wbf[:, 0:128].bitcast(BF16)
        W0_b = [wbf[:, 128 + 128 * i:256 + 128 * i].bitcast(BF16)
                for i in range(NUM_BLOCKS)]
        W1_b = [wbf[:, 640 + 128 * i:768 + 128 * i].bitcast(BF16)
                for i in range(NUM_BLOCKS)]
        Wc_fb = [wbf[:, 1152 + 128 * i:1280 + 128 * i].bitcast(BF16)
                 for i in range(NUM_BLOCKS)]
        Wo_b = wbf[:, 1664:1665].bitcast(BF16)
        Wp_xb = xbf[:, 0:128].bitcast(BF16)
        Wc_xb = [xbf[:, 128 + 128 * i:256 + 128 * i].bitcast(BF16)
                 for i in range(NUM_BLOCKS)]
        feW2_t = feW2_b[:].bitcast(BF16)
        bp_t = biasb[:, 0:1]
        g0_t = [biasb[:, 1 + i:2 + i] for i in range(NUM_BLOCKS)]
        b0_t = [biasb[:, 5 + i:6 + i] for i in range(NUM_BLOCKS)]
        g1_t = [biasb[:, 9 + i:10 + i] for i in range(NUM_BLOCKS)]
        comb1_t = [biasb[:, 13 + i:14 + i] for i in range(NUM_BLOCKS)]
        comb2_t = [biasb[:, 17 + i:18 + i] for i in range(NUM_BLOCKS)]
        gO_t = biasb[:, 21:22]
        bO_t = biasb[:, 22:23]
        bo_t = biasb[:, 23:24]
        feb1_t = biasb[0:FE_HID, 24:25]
        feb2_t = biasb[:, 25:26]

        # ---------- generic packed-knn helper ----------
        def packed_dist(lhs_t, rhs_t, contract, tagp):
            """matmul [contract,128] x [contract, 2048] -> centered S in PSUM,
            pack (bits & ~0xFFF) | ebits into an F32 SBUF tile [128, 2048]."""
            pk = big.tile([128, HALF], F32, tag=tagp, name=tagp)
            for ci in range(2):
                o = 1024 * ci
                pd = ps_d.tile([128, 1024], F32, tag="pd")
                for mj in range(2):
                    oo = 512 * mj
                    nc.tensor.matmul(pd[:, oo:oo + 512],
                                     lhs_t[:].bitcast(BF16),
                                     rhs_t[:, o + oo:o + oo + 512].bitcast(BF16),
                                     start=True, stop=True)
                nc.vector.scalar_tensor_tensor(
                    out=pk[:, o:o + 1024].bitcast(I32), in0=pd[:].bitcast(I32),
                    scalar=maskc[:, 0:1], in1=ebits[:, o:o + 1024],
                    op0=OP.bitwise_and, op1=OP.bitwise_or)
            return pk

        def topk_rounds(pk, rounds, tagv):
            """MAX8+MATCH_REPLACE rounds on [p, w] packed tile ->
            [p, 8*rounds] F32 packed top values (sorted within rounds)."""
            p, w = pk.shape[0], pk.shape[1]
            outv = con.tile([p, 8 * rounds], F32, tag=tagv, name=tagv)
            cur = pk
            for r in range(rounds):
                mx = sc.tile([p, 8], F32, tag="mx8")
                nc.vector.max(out=mx[:], in_=cur[:])
                nc.vector.tensor_copy(outv[:, 8 * r:8 * r + 8], mx[:])
                if r < rounds - 1:
                    nxt = dp.tile([p, w], F32, tag=f"{tagv}mr", name=f"{tagv}mr")
                    nc.vector.match_replace(
                        out=nxt[:], in_to_replace=mx[:],
                        in_values=cur[:], imm_value=NEG)
                    cur = nxt
            return outv

        def merge_extract(half_tops, k, tagm):
            """[128, k] per-half packed tops -> merged [64, k] indices I32."""
            mg = con.tile([QH, 2 * k], F32, tag=f"{tagm}mg", name=f"{tagm}mg")
            nc.sync.dma_start(mg[:, 0:k], half_tops[0:QH, :])
            nc.sync.dma_start(mg[:, k:2 * k], half_tops[QH:128, :])
            rounds = k // 8
            sel = con.tile([QH, k], F32, tag=f"{tagm}sel", name=f"{tagm}sel")
            cur = mg
            for r in range(rounds):
                mx = sc.tile([QH, 8], F32, tag="mx8")
                nc.vector.max(out=mx[:], in_=cur[:])
                nc.vector.tensor_copy(sel[:, 8 * r:8 * r + 8], mx[:])
                if r < rounds - 1:
                    nxt = sc.tile([QH, 2 * k], F32, tag=f"{tagm}mr",
                                  name=f"{tagm}mr")
                    nc.vector.match_replace(
                        out=nxt[:], in_to_replace=mx[:], in_values=cur[:],
                        imm_value=NEG)
                    cur = nxt
            idx = con.tile([QH, k], I32, tag=f"{tagm}idx", name=f"{tagm}idx")
            nc.vector.scalar_tensor_tensor(
                out=idx[:], in0=sel[:].bitcast(I32), scalar=maskc[0:QH, 1:2],
                in1=izer[0:QH, 0:k], op0=OP.bitwise_and, op1=OP.bitwise_or)
            return idx

        # ---------- feat knn (3D, k=16) ----------
        pkA = packed_dist(lhsA_t, rhsA_t, CA, "pkA")
        topsA = topk_rounds(pkA, 2, "tA")
        idx16 = merge_extract(topsA, 16, "fA")

        # EdgeConv: 8 pair-iterations over the 16 neighbors
        h_acc = con.tile([FE_HID, 128], F32)
        nc.vector.memset(h_acc[:], 0.0)
        for kk in range(8):
            it2 = sc.tile([128, 1], I32, tag="it2")
            nc.sync.dma_start(it2[0:QH, :], idx16[:, kk:kk + 1])
            nc.sync.dma_start(it2[QH:128, :], idx16[:, kk + 8:kk + 9])
            nb = sc.tile([128, 3], F32, tag="nb")
            nc.gpsimd.indirect_dma_start(
                out=nb[:], out_offset=None, in_=noisy[:],
                in_offset=bass.IndirectOffsetOnAxis(ap=it2[:, :1], axis=0))
            nbT_ps = ps_sm.tile([3, 128], F32, tag="pt")
            nc.tensor.transpose(nbT_ps[:], nb[:, 0:3], ident[:])
            dif = sc.tile([3, 128], F32, tag="dif")
            nc.vector.tensor_tensor(out=dif[:], in0=nbT_ps[:], in1=ctr2T[:],
                                    op=OP.subtract)
            hp = ps_sm.tile([FE_HID, 128], F32, tag="pt")
            nc.tensor.matmul(hp[:], feW1a, ctr2T[:], start=True, stop=False)
            nc.tensor.matmul(hp[:], feW1b, dif[:], start=False, stop=True)
            nc.vector.scalar_tensor_tensor(
                out=h_acc[:], in0=hp[:], scalar=feb1_t, in1=h_acc[:],
                op0=OP.add, op1=OP.max)
        hq = con.tile([FE_HID, QH], BF16)
        nc.vector.tensor_tensor(out=hq[:], in0=h_acc[:, 0:QH],
                                in1=h_acc[:, QH:128], op=OP.max)
        fps = ps_sm.tile([FEAT_DIM, QH], F32, tag="pt")
        nc.tensor.matmul(fps[:], feW2_t, hq[:], start=True, stop=True)
        featT = con.tile([FEAT_DIM, QH], BF16)
        nc.scalar.activation(featT[:], fps[:], AF.Relu, bias=feb2_t,
                             scale=1.0)

        # ---------- frame knn (xy, k=32) ----------
        pkC = packed_dist(lhsC_t, rhsC_t, CC, "pkC")
        topsC = topk_rounds(pkC, 4, "tC")
        nn32 = merge_extract(topsC, 32, "fC")

        # ---------- per-t: gather ntab, clean knn, gt; scorenet interleaved ----------
        xyzT = big.tile([3, QH * K_FRAME], BF16)
        zf = con.tile([128, 16], F32)
        gt = con.tile([128, 16], F32)
        lacc = con.tile([128, 1], F32)
        nc.vector.memset(lacc[:], 0.0)
        CH = [1024, 1024, 1024, 1024, 404]

        def t_chunk(t):
            it = sc.tile([128, 1], I32, tag="it")
            nc.sync.dma_start(it, nn32[4 * t:4 * t + 4, :])
            fsp = sc.tile([128, NTAB_W], F32, tag="fsp")
            nc.gpsimd.indirect_dma_start(
                out=fsp[:], out_offset=None, in_=ntab[:],
                in_offset=bass.IndirectOffsetOnAxis(ap=it[:, :1], axis=0))
            nc.scalar.copy(zf[:, t:t + 1], fsp[:, 2:3])
            pt1 = ps_sm.tile([3, 128], F32, tag="pt")
            nc.tensor.transpose(pt1[:], fsp[:, 0:3], ident[:])
            nc.vector.tensor_tensor(
                out=xyzT[:, 128 * t:128 * t + 128], in0=pt1[:],
                in1=_bcast_qk(ctrTk, 4 * t, 4, 32), op=OP.subtract)
            pt2 = ps_sm.tile([CD, 128], F32, tag="pt")
            nc.tensor.transpose(pt2[:], fsp[:, 4:4 + CD], ident[:])
            lhsD = sc.tile([CD, 128], BF16, tag="lhsD")
            nc.scalar.copy(lhsD[:], pt2[:])

            distD = dp.tile([128, N_CLEAN], F32, tag="dist")
            o = 0
            for w in CH:
                pd = ps_d.tile([128, 1024], F32, tag="pd")
                for oo in range(0, w, 512):
                    ww = min(512, w - oo)
                    nc.tensor.matmul(pd[:, oo:oo + ww], lhsD[:],
                                     cpl[:, o + oo:o + oo + ww].bitcast(BF16),
                                     start=True, stop=True)
                nc.scalar.copy(distD[:, o:o + w], pd[:, :w])
                o += w
            mxD = sc.tile([128, 8], F32, tag="mx8")
            nc.vector.max(out=mxD[:], in_=distD[:])
            zsum = sc.tile([128, 1], F32, tag="zsum")
            nc.vector.scalar_tensor_tensor(
                out=distD[:], in0=distD[:], scalar=mxD[:, 3:4], in1=czb[:],
                op0=OP.is_ge, op1=OP.mult, accum_out=zsum[:])
            nc.vector.scalar_tensor_tensor(
                out=gt[:, t:t + 1], in0=zsum[:], scalar=0.25, in1=zf[:, t:t + 1],
                op0=OP.mult, op1=OP.subtract)

        def score_blk(blk):
            r0 = 512 * blk
            xyz_b = xyzT[:, r0:r0 + 512]
            feat_b = _bcast_qk(featT, 16 * blk, 16, 32)
            pA = ps_net.tile([HID, 512], F32, tag="ps")
            nc.tensor.matmul(pA[:], Wp_xb, xyz_b, start=True, stop=False)
            nc.tensor.matmul(pA[:], Wp_fb, feat_b, start=False, stop=True)
            net = sc.tile([HID, 512], F32, tag="net")
            nc.scalar.activation(net[:], pA[:], AF.Identity, bias=bp_t,
                                 scale=1.0)
            for i in range(NUM_BLOCKS):
                h1 = sc.tile([HID, 512], BF16, tag="h1")
                nc.scalar.activation(h1[:], net[:], AF.Relu, bias=b0_t[i],
                                     scale=g0_t[i])
                pB = ps_net.tile([HID, 512], F32, tag="ps")
                nc.tensor.matmul(pB[:], W0_b[i], h1[:], start=True, stop=True)
                h2 = sc.tile([HID, 512], BF16, tag="h1")
                nc.scalar.activation(h2[:], pB[:], AF.Relu, bias=comb1_t[i],
                                     scale=g1_t[i])
                pC = ps_net.tile([HID, 512], F32, tag="ps")
                nc.tensor.matmul(pC[:], W1_b[i], h2[:], start=True, stop=False)
                nc.tensor.matmul(pC[:], Wc_xb[i], xyz_b, start=False, stop=False)
                nc.tensor.matmul(pC[:], Wc_fb[i], feat_b, start=False, stop=True)
                nc.vector.scalar_tensor_tensor(
                    out=net[:], in0=pC[:], scalar=comb2_t[i], in1=net[:],
                    op0=OP.add, op1=OP.add)
            rfin = sc.tile([HID, 512], BF16, tag="h1")
            nc.scalar.activation(rfin[:], net[:], AF.Relu, bias=bO_t,
                                 scale=gO_t)
            for j in range(4):
                t = 4 * blk + j
                gp_ps = ps_sm.tile([128, 1], F32, tag="pt")
                nc.tensor.matmul(gp_ps[:], rfin[:, 128 * j:128 * j + 128],
                                 Wo_b, start=True, stop=True)
                diff = sc.tile([128, 1], F32, tag="diff")
                nc.vector.scalar_tensor_tensor(
                    out=diff[:], in0=gp_ps[:], scalar=bo_t,
                    in1=gt[:, t:t + 1], op0=OP.add, op1=OP.subtract)
                sq = sc.tile([128, 1], F32, tag="sq")
                nc.vector.tensor_tensor(out=sq[:], in0=diff[:], in1=diff[:],
                                        op=OP.mult)
                nc.vector.tensor_tensor(out=lacc[:], in0=lacc[:], in1=sq[:],
                                        op=OP.add)

        for blk in range(4):
            for tt in range(4):
                t_chunk(4 * blk + tt)
            score_blk(blk)

        if DEBUG:
            nc.sync.dma_start(idx16_d[:], idx16[:])
            nc.sync.dma_start(nn32_d[:], nn32[:])
            nc.sync.dma_start(gt_d[:], gt[:])
            fdbg = con.tile([FEAT_DIM, QH], F32)
            nc.vector.tensor_copy(fdbg[:], featT[:])
            nc.sync.dma_start(feat_d[:], fdbg[:])

        lps = ps_sm.tile([1, 1], F32, tag="pt")
        nc.tensor.matmul(lps[:], lacc[:], ones[:], start=True, stop=True)
        lsb = con.tile([1, 1], F32)
        nc.scalar.copy(lsb[:], lps[:])
        nc.sync.dma_start(partial[:], lsb[:])

    _split_multi_waits(nc)
    return nc


# ---------------- host-side preprocessing ----------------

def _bf16_rne(x):
    x = np.asarray(x, np.float32)
    u = x.view(np.uint32)
    r = ((u >> 16) + ((u >> 15) & 1)).astype(np.uint32) << 16
    out = r.view(np.float32).copy()
    return out


def _split3(x):
    x = np.asarray(x, np.float32)
    h = _bf16_rne(x)
    m = _bf16_rne(x - h)
    l = _bf16_rne(x - h - m)
    return h, m, l


def _f32_to_bf16_bits(x):
    x = np.ascontiguousarray(np.asarray(x, np.float32))
    u = x.view(np.uint32)
    r = ((u >> 16) + ((u >> 15) & 1)).astype(np.uint32)
    return (r & 0xFFFF).astype(np.uint16)


def _plane_rows(coords, c2):
    """c-side rows for a set of candidate points.
    coords: (m, d); c2: (m,) the squared-norm term.
    Returns list of (m,) f32 rows: per dim [ch, cm, cl, ch, cm, ch],
    then [c2h, c2m, c2l]."""
    rows = []
    for d in range(coords.shape[1]):
        ch, cm, cl = _split3(coords[:, d])
        rows += [ch, cm, cl, ch, cm, ch]
    c2h, c2m, c2l = _split3(c2)
    rows += [c2h, c2m, c2l]
    return rows


def _q_rows(q, g):
    """q-side rows for queries. q: (n, d) pre-doubled coords; g: (n,)
    center term. Per dim [th, th, th, tm, tm, tl], then [-1,-1,-1],
    then [gh, gm, gl]."""
    n = q.shape[0]
    rows = []
    for d in range(q.shape[1]):
        th, tm, tl = _split3(q[:, d])
        rows += [th, th, th, tm, tm, tl]
    neg1 = np.full(n, -1.0, np.float32)
    rows += [neg1, neg1, neg1]
    gh, gm, gl = _split3(g)
    rows += [gh, gm, gl]
    return rows


def _pair_c_rows(coords, c2):
    """c-side rows matching _q_rows ordering: per-dim pairs + c2 rows +
    center-const rows [1,1,1]."""
    rows = _plane_rows(coords, c2)
    m = coords.shape[0]
    one = np.ones(m, np.float32)
    rows += [one, one, one]
    return rows


def build_in_maps(inputs):
    pcl_noisy = np.ascontiguousarray(np.asarray(inputs["pcl_noisy"], np.float32))
    pcl_clean = np.ascontiguousarray(np.asarray(inputs["pcl_clean"], np.float32))
    pnt_idx = np.asarray(inputs["pnt_idx"]).astype(np.int64)

    common = {"ident": np.eye(128, dtype=np.float32),
              "maskc": np.tile(np.array([[~0xFFF, 0xFFF]], np.int32), (128, 1))}
    W = {k: np.asarray(inputs[k], np.float32) for k in
         ("feW1", "feb1", "feW2", "feb2", "Wp", "bp", "g0", "b0", "W0",
          "bf0", "g1", "b1", "W1", "bf1", "Wc", "bc", "gO", "bO", "Wo", "bo")}
    common["feW1"] = np.ascontiguousarray(W["feW1"])
    common["few2b"] = _f32_to_bf16_bits(W["feW2"])
    wbf = np.zeros((128, 1665), np.float32)
    wbf[:, 0:128] = W["Wp"][3:, :]
    for i in range(NUM_BLOCKS):
        wbf[:, 128 + 128 * i:256 + 128 * i] = W["W0"][i]
        wbf[:, 640 + 128 * i:768 + 128 * i] = W["W1"][i]
        wbf[:, 1152 + 128 * i:1280 + 128 * i] = W["Wc"][i, 3:, :]
    wbf[:, 1664] = W["Wo"][:, 0]
    common["wbf"] = _f32_to_bf16_bits(wbf)
    xbf = np.zeros((3, 640), np.float32)
    xbf[:, 0:128] = W["Wp"][0:3, :]
    for i in range(NUM_BLOCKS):
        xbf[:, 128 + 128 * i:256 + 128 * i] = W["Wc"][i, 0:3, :]
    common["xbf"] = _f32_to_bf16_bits(xbf)
    bb = np.zeros((128, 26), np.float32)
    bb[:, 0] = W["bp"]
    for i in range(NUM_BLOCKS):
        bb[:, 1 + i] = W["g0"][i]
        bb[:, 5 + i] = W["b0"][i]
        bb[:, 9 + i] = W["g1"][i]
        bb[:, 13 + i] = W["g1"][i] * W["bf0"][i] + W["b1"][i]
        bb[:, 17 + i] = W["bf1"][i] + W["bc"][i]
    bb[:, 21] = W["gO"]
    bb[:, 22] = W["bO"]
    bb[:, 23] = W["bo"][0]
    bb[0:FE_HID, 24] = W["feb1"]
    bb[:, 25] = W["feb2"]
    common["biasb"] = bb
    eb = np.empty((128, HALF), np.int32)
    eb[0:64, :] = np.arange(HALF, dtype=np.int32)[None, :]
    eb[64:128, :] = np.arange(HALF, dtype=np.int32)[None, :] + HALF
    common["ebits"] = eb

    in_maps = []
    for core in range(8):
        b, h = core // 2, core % 2
        m = dict(common)
        pn = pcl_noisy[b]                       # (4096, 3)
        pc = pcl_clean[b]                       # (4500, 3)
        q = pn[pnt_idx[64 * h:64 * h + 64]]     # (64, 3)

        m["noisy"] = pn
        # ntab: per noisy point the clean-dist stationary rows
        tq2d = 2.0 * pn[:, 0:2]
        g_clean = -(pn[:, 0].astype(np.float64) ** 2
                    + pn[:, 1].astype(np.float64) ** 2).astype(np.float32)
        qr = _q_rows(tq2d, g_clean)             # 18 rows of (4096,)
        ntab = np.zeros((N_NOISY, NTAB_W), np.float32)
        ntab[:, 0:3] = pn
        for r, row in enumerate(qr):
            ntab[:, 4 + r] = row
        m["ntab"] = ntab

        # clean planes (c-side): xy dims + c2(xy) + const-1 center rows
        c2xy = (pc[:, 0].astype(np.float64) ** 2
                + pc[:, 1].astype(np.float64) ** 2).astype(np.float32)
        crows = _pair_c_rows(pc[:, 0:2], c2xy)  # 18 rows of (4500,)
        m["cplanes"] = _f32_to_bf16_bits(np.stack(crows))

        # feat dist (3D): rhs interleaved halves, lhs dup-half
        n2 = (pn.astype(np.float64) ** 2).sum(1).astype(np.float32)
        rhsA = np.zeros((CA, HALF), np.float32)
        lhsA = np.zeros((CA, 128), np.float32)
        q2_3d = (q.astype(np.float64) ** 2).sum(1).astype(np.float32)
        qrA = _q_rows(2.0 * q, -(q2_3d + 1.0))  # 21+3=24 rows of (64,)
        for half in range(2):
            sl = slice(half * HALF, (half + 1) * HALF)
            cr = _pair_c_rows(pn[sl, 0:3], n2[sl])   # 24 rows of (2048,)
            r0 = half * CA_H
            for r, row in enumerate(cr):
                rhsA[r0 + r, :] = row
            for r, row in enumerate(qrA):
                lhsA[r0 + r, half * 64:half * 64 + 64] = row
        m["rhsA"] = _f32_to_bf16_bits(rhsA)
        m["lhsA"] = _f32_to_bf16_bits(lhsA)

        # frame dist (2D)
        n2xy = (pn[:, 0:2].astype(np.float64) ** 2).sum(1).astype(np.float32)
        rhsC = np.zeros((CC, HALF), np.float32)
        lhsC = np.zeros((CC, 128), np.float32)
        q2_2d = (q[:, 0:2].astype(np.float64) ** 2).sum(1).astype(np.float32)
        qrC = _q_rows(2.0 * q[:, 0:2], -(q2_2d + 1.0))  # 15+3=18 rows
        for half in range(2):
            sl = slice(half * HALF, (half + 1) * HALF)
            cr = _pair_c_rows(pn[sl, 0:2], n2xy[sl])     # 18 rows
            r0 = half * CC_H
            for r, row in enumerate(cr):
                rhsC[r0 + r, :] = row
            for r, row in enumerate(qrC):
                lhsC[r0 + r, half * 64:half * 64 + 64] = row
        m["rhsC"] = _f32_to_bf16_bits(rhsC)
        m["lhsC"] = _f32_to_bf16_bits(lhsC)

        m["czb"] = np.ascontiguousarray(
            np.broadcast_to(pc[:, 2][None, :], (128, N_CLEAN)))
        m["ctrTk"] = np.ascontiguousarray(q.T)           # (3, 64)
        ctr2 = np.concatenate([q, q], axis=0)            # (128, 3)
        m["ctr2T"] = np.ascontiguousarray(ctr2.T)        # (3, 128)
        in_maps.append(m)
    return in_maps


_NC_CACHE = {}


def _get_nc():
    if "nc" not in _NC_CACHE:
        _NC_CACHE["nc"] = _build()
    return _NC_CACHE["nc"]


def kernel(**inputs) -> np.ndarray:
    from concourse.bass_utils import run_bass_kernel_spmd

    in_maps = build_in_maps(inputs)
    res = run_bass_kernel_spmd(_get_nc(), in_maps, core_ids=list(range(8)))
    total = float(np.sum([np.asarray(res.results[i]["partial"]).reshape(())
                          for i in range(8)]))
    loss = 0.5 * total / (B * N_TRAIN * K_FRAME) / DSM_SIGMA
    return np.float32(loss)
